# revision 8
# baseline (speedup 1.0000x reference)
"""Trainium2 Bass kernel for LyapunovSDELayer.

Reference computes, per batch element b with lam0 = current_lyapunov[b, 0]:
    path[b, 0] = lam0
    path[b, t] = clip(path[b, t-1] + KAPPA*(THETA - path[b, t-1]), 0, 1)

The step map is affine: lam -> (1-KAPPA)*lam + KAPPA*THETA with
(1-KAPPA) = 0.5 exactly, and for lam0 in [0, 1) the iterates stay inside
[0.15, 0.65] so the clip never binds.  Hence

    path[b, t] = THETA + 0.5**t * (lam0 - THETA)

0.5**t is a power of two, so the device computation
    fl(THETA + fl(w_t * fl(lam0 - THETA)))
matches the reference fp32 scan to ~1 ulp (max rel err ~1e-7, verified).

The kernel is a pure memory-bound broadcast: each core computes its
16384x256 fp32 output shard (16 MB) as an outer product
    out[p*R + r, t] = w[t] * d[p, r] + THETA
with batch on SBUF partitions and (row-in-partition, time) on the free
dim, so every DMA store is 128 contiguous per-partition runs.
"""

import numpy as np

import concourse.bacc as bacc
import concourse.bass as bass
import concourse.mybir as mybir
from concourse.tile import TileContext
from concourse.bass_utils import run_bass_kernel_spmd

THETA = 0.3
KAPPA = 0.5
N_CORES = 8
P = 128  # SBUF partitions

# module-level cache: (batch_per_core, horizon, groups_per_chunk) -> Bass
_NC_CACHE = {}

# test harness hook: set by test.py to capture BassKernelResults
LAST_RESULTS = None
TRACE = False


def _build_nc(bpc: int, horizon: int, G: int) -> bass.Bass:
    """Build the per-core Bass module.

    Inputs (per core):
      lam [P, R]  fp32 : lam0 shard reshaped; lam[p, r] = lam0[p*R + r]
      wt  [P, H]  fp32 : wt[p, t] = 0.5**t (broadcast across partitions)
    Output:
      out [bpc, H] fp32: the path shard
    """
    R = bpc // P
    assert R * P == bpc
    assert R % G == 0
    n_chunks = R // G
    H = horizon
    f32 = mybir.dt.float32

    # Bacc (not raw Bass): its compile pipeline splits multi-sem waits
    # into EventSemaphore instructions (TRN2 encodes at most one wait per
    # compute instruction).
    nc = bacc.Bacc()
    lam = nc.dram_tensor("lam", [P, R], f32, kind="ExternalInput")
    wt = nc.dram_tensor("wt", [P, H], f32, kind="ExternalInput")
    out = nc.dram_tensor("out", [bpc, H], f32, kind="ExternalOutput")
    # [bpc, H] -> [P, R*H]; partition p's free dim is contiguous in DRAM
    out_v = out[:, :].rearrange("(p r) t -> p (r t)", p=P)

    # Constraints honored here (ISA: ACT/DVE encode at most ONE semaphore
    # wait per instruction; Tile attaches waits to instructions directly):
    #  - each output tile is written by exactly one compute engine and
    #    read by one DMA (1 recycle wait max on the slot's first writer)
    #  - each engine computes its own d = lam0 - THETA (no cross-engine
    #    dependency on the first main op: it only waits on the wt load)
    #  - no overwrites of already-written tile bytes (same-engine bank
    #    overlap would add an own-engine sem wait).  t=0 needs no fixup:
    #    w[0] = 1 and fl(THETA + fl(lam0-THETA)) == lam0 exactly for
    #    inputs on the 2^-24 uniform grid.
    Gh = G // 2  # groups per engine per chunk
    with TileContext(nc) as tc:
        with (
            tc.tile_pool(name="const", bufs=1) as cpool,
            tc.tile_pool(name="dve", bufs=3) as dpool,
            tc.tile_pool(name="act", bufs=3) as apool,
        ):
            wt_sb = cpool.tile([P, H], f32)
            nc.sync.dma_start(out=wt_sb, in_=wt[:, :])
            lam_sb = cpool.tile([P, R], f32)
            nc.sync.dma_start(out=lam_sb, in_=lam[:, :])
            d_dve = cpool.tile([P, R], f32)
            nc.vector.tensor_scalar(
                out=d_dve,
                in0=lam_sb,
                scalar1=THETA,
                scalar2=None,
                op0=mybir.AluOpType.subtract,
            )
            d_act = cpool.tile([P, R], f32)
            nc.scalar.activation(
                out=d_act,
                in_=lam_sb,
                func=mybir.ActivationFunctionType.Copy,
                bias=-THETA,
                scale=1.0,
            )

            for c in range(n_chunks):
                # DVE half: groups [c*G, c*G+Gh)
                dt = dpool.tile([P, Gh * H], f32)
                for g in range(Gh):
                    r = c * G + g
                    nc.vector.tensor_scalar(
                        out=dt[:, g * H : (g + 1) * H],
                        in0=wt_sb,
                        scalar1=d_dve[:, r : r + 1],
                        scalar2=THETA,
                        op0=mybir.AluOpType.mult,
                        op1=mybir.AluOpType.add,
                    )
                nc.sync.dma_start(
                    out=out_v[:, (c * G) * H : (c * G + Gh) * H], in_=dt
                )

                # ACT half: groups [c*G+Gh, (c+1)*G)
                at = apool.tile([P, Gh * H], f32)
                for g in range(Gh):
                    r = c * G + Gh + g
                    nc.scalar.activation(
                        out=at[:, g * H : (g + 1) * H],
                        in_=wt_sb,
                        func=mybir.ActivationFunctionType.Copy,
                        bias=THETA,
                        scale=d_act[:, r : r + 1],
                    )
                nc.sync.dma_start(
                    out=out_v[:, (c * G + Gh) * H : ((c + 1) * G) * H], in_=at
                )
    # Run the bacc compile pipeline (register allocation, event-semaphore
    # wait splitting, ...); run_bass_via_pjrt does not call finalize.
    nc.finalize()
    return nc


def kernel(current_lyapunov: np.ndarray, horizon) -> np.ndarray:
    global LAST_RESULTS
    lam0 = np.ascontiguousarray(np.asarray(current_lyapunov, np.float32)).reshape(-1)
    H = int(horizon)
    B = lam0.shape[0]
    assert B % (N_CORES * P) == 0, B
    bpc = B // N_CORES
    R = bpc // P
    G = 16
    while R % G:
        G //= 2

    key = (bpc, H, G)
    if key not in _NC_CACHE:
        _NC_CACHE[key] = _build_nc(bpc, H, G)
    nc = _NC_CACHE[key]

    # 0.5**t exact in fp64, cast to fp32 (exact for t<=149, 0 below; the
    # tail underflow is invisible: THETA + tiny rounds to THETA anyway)
    w = (0.5 ** np.arange(H, dtype=np.float64)).astype(np.float32)
    wt_full = np.ascontiguousarray(np.broadcast_to(w, (P, H)))

    in_maps = []
    for c in range(N_CORES):
        shard = lam0[c * bpc : (c + 1) * bpc]
        in_maps.append({"lam": shard.reshape(P, R), "wt": wt_full})

    res = run_bass_kernel_spmd(
        nc,
        in_maps,
        core_ids=list(range(N_CORES)),
        trace=TRACE,
    )
    LAST_RESULTS = res
    return np.concatenate([r["out"] for r in res.results], axis=0)
